# revision 4
# baseline (speedup 1.0000x reference)
"""GQA attention kernel for 8 Trainium2 NeuronCores.

Sharding: core c -> (b = c // 4, kv-group gk = c % 4).
Each core computes, for its batch b and its kv head gk (which owns the 4
contiguous q-heads gk*4..gk*4+3):
    q/k/v projections, attention, and a partial out-projection
    out_partial[b] = o_heads @ Wo[:, gk*512:(gk+1)*512].T
Host sums the 4 partials per batch (bf16 partials, f32 accumulate).

All matmuls in bf16 (fp32 PSUM accumulation). Softmax without max
subtraction (scores are bounded ~|4.5| at this problem's weight scale);
row sums come free from a ones-column appended to V; normalization is
applied to the 128-wide per-head output ahead of the out projection.

Layout (per core), everything E/K-major for the PE:
  xT  [E, N]   = x[b].T          kT [128d, N]    scoresT [s, n] chunks
  wq  [E, 512] = Wq rows.T       qT [128, 4g, N]
  wk  [E, 128] = Wk rows.T       v  [128, 16st, 130] (col 128 = ones)
  wv  [E, 128]                   oT [128, 4g, N]
  wo  [512, E] = Wo cols.T       out [N, E] bf16 partial

Schedule notes (from NTFF profiling):
 - input DMAs are issued in consumption order on separate queues so the
   first k matmul can start ~11us instead of ~25us;
 - dummy matmuls on the identity ramp the PE out of its low p-state
   while the first x chunk is in flight;
 - o-groups trail their chunk's exps by >=2 score-pairs so the PE never
   waits on the ACT engine; out-projection n-tiles are spread one per
   sub-slot instead of 4-tile bursts (P1/DVE ring pressure);
 - out DMAs are issued from gpsimd (keeps the ACT engine exp-only).
"""

import sys

sys.path.insert(0, "/opt/trn_rl_repo")

import numpy as np
import ml_dtypes

import concourse.bass as bass
import concourse.mybir as mybir
import concourse.tile as tile
from concourse import bacc
from concourse.bass_utils import run_bass_kernel_spmd
from concourse.masks import make_identity

BF16 = mybir.dt.bfloat16
F32 = mybir.dt.float32
bf16 = ml_dtypes.bfloat16

B, N, E = 2, 2048, 2048
H, D, G = 16, 128, 4
HKV = H // G
JL = G * D                     # 512 local q-head dims per core
ET = E // 128                  # 16
NT = N // 128                  # 16
CH = N // 512                  # 4
SCALE = 1.0 / float(np.sqrt(D))

_cached = {}


def _build(iters=1):
    nc = bacc.Bacc("TRN2", target_bir_lowering=False, debug=False, num_devices=8)

    xT = nc.dram_tensor("xT", [E, N], BF16, kind="ExternalInput")
    wq = nc.dram_tensor("wq", [E, JL], BF16, kind="ExternalInput")
    wk = nc.dram_tensor("wk", [E, D], BF16, kind="ExternalInput")
    wv = nc.dram_tensor("wv", [E, D], BF16, kind="ExternalInput")
    wo = nc.dram_tensor("wo", [JL, E], BF16, kind="ExternalInput")
    out = nc.dram_tensor("out", [N, E], BF16, kind="ExternalOutput")

    with tile.TileContext(nc) as tc:
        with (
            tc.tile_pool(name="const", bufs=1) as cpool,
            tc.tile_pool(name="xp", bufs=1) as xpool,
            tc.tile_pool(name="wp", bufs=1) as wpool,
            tc.tile_pool(name="kvp", bufs=1) as kvpool,
            tc.tile_pool(name="qp", bufs=1) as qpool,
            tc.tile_pool(name="pp", bufs=3) as ppool,
            tc.tile_pool(name="op", bufs=4) as opool,
            tc.tile_pool(name="otp", bufs=1) as otpool,
            tc.tile_pool(name="outp", bufs=3) as outpool,
            tc.tile_pool(name="ps1", bufs=2, space="PSUM") as P1,
            tc.tile_pool(name="ps2", bufs=4, space="PSUM") as P2,
        ):
            ident = cpool.tile([128, 128], BF16, tag="ident")
            make_identity(nc, ident[:])

            for _ in range(iters):
                _emit_iter(nc, tc, ident, xpool, wpool, kvpool, qpool, ppool,
                           opool, otpool, outpool, P1, P2,
                           xT, wq, wk, wv, wo, out)

    nc.compile()
    return nc


def _emit_iter(nc, tc, ident, xpool, wpool, kvpool, qpool, ppool, opool,
               otpool, outpool, P1, P2, xT, wq, wk, wv, wo, out):
    x_sb = xpool.tile([128, ET, N], BF16, tag="x")
    wq_sb = wpool.tile([128, ET, JL], BF16, tag="wq")
    wk_sb = wpool.tile([128, ET, D], BF16, tag="wk")
    wv_sb = wpool.tile([128, ET, D], BF16, tag="wv")
    wo_sb = wpool.tile([128, G, E], BF16, tag="wo")
    kT_sb = kvpool.tile([128, N], BF16, tag="kT")
    v_sb = kvpool.tile([128, NT, 130], BF16, tag="v")
    qT_sb = qpool.tile([128, G, N], BF16, tag="qT")
    oT_sb = otpool.tile([128, G, N], BF16, tag="oT")

    # --- PE warmup: dummy matmuls on the identity while DMAs are in
    # flight, so the PE reaches its full p-state before real work.
    for i in range(24):
        wmt = P2.tile([128, 130], F32, tag="oc", name=f"warm{i}")
        nc.tensor.matmul(wmt[:, 0:128], ident[:], ident[:], start=True, stop=True)

    # --- input DMAs, in consumption order (queues: sync/scalar/gpsimd) ---
    # wk first (2 pieces, sync queue); k's et-loop starts on piece 0.
    wkr = wk.rearrange("(a p) d -> p a d", p=128)
    nc.sync.dma_start(wk_sb[:, 0:8, :], wkr[:, 0:8, :])
    nc.sync.dma_start(wk_sb[:, 8:16, :], wkr[:, 8:16, :])
    # wv on gpsimd ahead of its x share.
    nc.gpsimd.dma_start(wv_sb[:], wv.rearrange("(a p) d -> p a d", p=128))
    # x: 16 et-chunks of 0.5MB over 3 queues, byte-balanced (sync carries
    # wk so it gets fewer), arrival ~= et order.
    xr = xT.rearrange("(a p) n -> p a n", p=128)
    xengs = {0: nc.scalar, 1: nc.gpsimd, 2: nc.scalar, 3: nc.gpsimd,
             4: nc.scalar, 5: nc.gpsimd, 6: nc.sync, 7: nc.scalar,
             8: nc.gpsimd, 9: nc.sync, 10: nc.scalar, 11: nc.gpsimd,
             12: nc.sync, 13: nc.scalar, 14: nc.gpsimd, 15: nc.sync}
    for et in range(ET):
        xengs[et].dma_start(x_sb[:, et, :], xr[:, et, :])
    # wq in 4 per-head pieces behind x on sync: v-tails run before the
    # q-pairs, so piece g only needs to land ~(47 + 10g)us in.
    wqr = wq.rearrange("(a p) j -> p a j", p=128)
    for g in range(G):
        nc.sync.dma_start(wq_sb[:, :, g * 128:(g + 1) * 128],
                          wqr[:, :, g * 128:(g + 1) * 128])
    # wo last on sync (needed ~halfway into attention).
    for jt in range(G):
        nc.sync.dma_start(wo_sb[:, jt, :], wo[jt * 128:(jt + 1) * 128, :])

    nc.vector.memset(v_sb[:, :, 128:129], 1.0)

    # --- phase 1 ---
    # kT: 4 chunk accumulators (2 double-bank P1 tiles) so the PE can trail
    # the x DMAs; v head tiles on P2 meanwhile.
    kp = [P1.tile([128, 1024], F32, tag="mm1024", name=f"kp{_i}") for _i in range(2)]
    kps = [kp[_i // 2][:, (_i % 2) * 512:(_i % 2 + 1) * 512] for _i in range(CH)]
    vps = [P2.tile([128, 130], F32, tag="oc", name=f"vps{_i}") for _i in range(4)]
    for et in range(ET):
        for sc in range(CH):
            nc.tensor.matmul(
                kps[sc], wk_sb[:, et, :], x_sb[:, et, sc * 512:(sc + 1) * 512],
                start=(et == 0), stop=(et == ET - 1),
            )
        for st in range(4):
            nc.tensor.matmul(
                vps[st][:, 0:128], x_sb[:, et, st * 128:(st + 1) * 128],
                wv_sb[:, et, :],
                start=(et == 0), stop=(et == ET - 1),
            )
    for sc in range(CH):
        nc.vector.tensor_copy(kT_sb[:, sc * 512:(sc + 1) * 512], kps[sc])
    for st in range(4):
        nc.vector.tensor_copy(v_sb[:, st, 0:128], vps[st][:, 0:128])

    # remaining v tiles interleaved with paired q groups
    def emit_v(st):
        ps = P2.tile([128, 130], F32, tag="oc")
        for et in range(ET):
            nc.tensor.matmul(
                ps[:, 0:128], x_sb[:, et, st * 128:(st + 1) * 128],
                wv_sb[:, et, :],
                start=(et == 0), stop=(et == ET - 1),
            )
        nc.vector.tensor_copy(v_sb[:, st, 0:128], ps[:, 0:128])

    def emit_q_pair(q0, q1):
        ps = P1.tile([128, 1024], F32, tag="mm1024")
        for half, (g, ncg) in enumerate((q0, q1)):
            sl = ps[:, half * 512:(half + 1) * 512]
            for et in range(ET):
                nc.tensor.matmul(
                    sl, wq_sb[:, et, g * 128:(g + 1) * 128],
                    x_sb[:, et, ncg * 512:(ncg + 1) * 512],
                    start=(et == 0), stop=(et == ET - 1),
                )
            nc.vector.tensor_copy(qT_sb[:, g, ncg * 512:(ncg + 1) * 512], sl)

    # v-tails first: they need only x, covering the window where wq is
    # still streaming in behind x.
    for st in range(4, NT):
        emit_v(st)
    qlist = [(g, ncg) for g in range(G) for ncg in range(CH)]
    for i in range(8):
        emit_q_pair(qlist[2 * i], qlist[2 * i + 1])

    # --- phase 2 + 3, pipelined per chunk of 512 n-columns ---
    # Scores for two s-tiles share one double-bank psum tile so a single
    # (wider, cheaper per element) Exp covers both. o-groups trail their
    # chunk by >= 2 score-pairs so the PE never catches the ACT engine;
    # one out-projection n-tile is emitted per sub-slot once a column's
    # four heads are done.
    def emit_score_pair(g, c, p_t, sp):
        ps = P1.tile([128, 1024], F32, tag="mm1024")
        for half in range(2):
            st = 2 * sp + half
            nc.tensor.matmul(
                ps[:, half * 512:(half + 1) * 512],
                kT_sb[:, st * 128:(st + 1) * 128],
                qT_sb[:, g, c * 512:(c + 1) * 512],
                start=True, stop=True,
            )
        nc.scalar.activation(
            p_t[:, 2 * sp * 512:(2 * sp + 2) * 512], ps[:],
            mybir.ActivationFunctionType.Exp, scale=SCALE,
        )

    def emit_o_group(g, c, p_t, t):
        pso = P2.tile([128, 130], F32, tag="oc")
        for st in range(NT):
            nc.tensor.matmul(
                pso[:, 0:129], p_t[:, st * 512 + t * 128: st * 512 + (t + 1) * 128],
                v_sb[:, st, 0:129],
                start=(st == 0), stop=(st == NT - 1),
            )
        rc = opool.tile([128, 1], F32, tag="recip")
        nc.vector.reciprocal(rc[:], pso[:, 128:129])
        o_n = opool.tile([128, 128], BF16, tag="o_n")
        nc.vector.tensor_scalar_mul(o_n[:], pso[:, 0:128], rc[:])
        pst = P2.tile([128, 128], BF16, tag="oc")
        nc.tensor.transpose(pst[:], o_n[:], ident[:])
        nc.vector.tensor_copy(
            oT_sb[:, g, c * 512 + t * 128: c * 512 + (t + 1) * 128], pst[:],
        )

    def emit_out_nt(nt):
        stage = outpool.tile([128, 2048], BF16, tag="out")
        for half in range(2):
            ps = P1.tile([128, 1024], F32, tag="mm1024")
            for e2 in range(2):
                ec = half * 2 + e2
                for g in range(G):
                    nc.tensor.matmul(
                        ps[:, e2 * 512:(e2 + 1) * 512],
                        oT_sb[:, g, nt * 128:(nt + 1) * 128],
                        wo_sb[:, g, ec * 512:(ec + 1) * 512],
                        start=(g == 0), stop=(g == G - 1),
                    )
            nc.vector.tensor_copy(stage[:, half * 1024:(half + 1) * 1024], ps[:])
            nc.gpsimd.dma_start(
                out[nt * 128:(nt + 1) * 128, half * 1024:(half + 1) * 1024],
                stage[:, half * 1024:(half + 1) * 1024],
            )

    # pending o-group / out-tile work queue: each entry is emitted at one
    # "slot" (after a score pair), keeping >= 2 pairs of exp lead.
    chunks = [(c, g) for c in range(CH) for g in range(G)]
    pending = []

    def pop_slot():
        if pending:
            pending.pop(0)()

    for i, (c, g) in enumerate(chunks):
        p_t = ppool.tile([128, NT * 512], BF16, tag="p", name=f"p{i}")
        for sub in range(4):
            emit_score_pair(g, c, p_t, 2 * sub)
            emit_score_pair(g, c, p_t, 2 * sub + 1)
            pop_slot()
        # queue this chunk's o-groups (consumed during the next chunk)
        for t in range(4):
            def oj(g=g, c=c, p_t=p_t, t=t, last=(g == G - 1)):
                emit_o_group(g, c, p_t, t)
                if last:
                    emit_out_nt(4 * c + t)
            pending.append(oj)
    while pending:
        pop_slot()


def get_nc(iters=1):
    key = ("nc", iters)
    if key not in _cached:
        _cached[key] = _build(iters)
    return _cached[key]


def make_in_maps(x, Wq, Wk, Wv, Wo):
    """Per-core host-side sharding. Core c -> (b=c//4, gk=c%4)."""
    in_maps = []
    xT = [np.ascontiguousarray(x[b].T).astype(bf16) for b in range(B)]
    wq_s = [np.ascontiguousarray(Wq[gk * JL:(gk + 1) * JL, :].T).astype(bf16)
            for gk in range(HKV)]
    wk_s = [np.ascontiguousarray(Wk[gk * D:(gk + 1) * D, :].T).astype(bf16)
            for gk in range(HKV)]
    wv_s = [np.ascontiguousarray(Wv[gk * D:(gk + 1) * D, :].T).astype(bf16)
            for gk in range(HKV)]
    wo_s = [np.ascontiguousarray(Wo[:, gk * JL:(gk + 1) * JL].T).astype(bf16)
            for gk in range(HKV)]
    for c in range(8):
        b, gk = c // 4, c % 4
        in_maps.append({
            "xT": xT[b], "wq": wq_s[gk], "wk": wk_s[gk],
            "wv": wv_s[gk], "wo": wo_s[gk],
        })
    return in_maps


def kernel(x, Wq, Wk, Wv, Wo):
    nc = get_nc()
    in_maps = make_in_maps(x, Wq, Wk, Wv, Wo)
    res = run_bass_kernel_spmd(nc, in_maps, core_ids=list(range(8)))
    out = np.empty((B, N, E), np.float32)
    for b in range(B):
        acc = res.results[b * 4]["out"].astype(np.float32)
        for gk in range(1, HKV):
            acc = acc + res.results[b * 4 + gk]["out"].astype(np.float32)
        out[b] = acc
    return out


# revision 8
# speedup vs baseline: 1.0273x; 1.0273x over previous
"""GQA attention kernel for 8 Trainium2 NeuronCores.

Sharding: core c -> (b = c // 4, kv-group gk = c % 4).
Each core computes, for its batch b and its kv head gk (which owns the 4
contiguous q-heads gk*4..gk*4+3):
    q/k/v projections, attention, and a partial out-projection
    out_partial[b] = o_heads @ Wo[:, gk*512:(gk+1)*512].T
Host sums the 4 partials per batch (bf16 partials, f32 accumulate).

All matmuls in bf16 (fp32 PSUM accumulation). Softmax without max
subtraction (scores are bounded ~|4.5| at this problem's weight scale);
row sums come free from a ones-column appended to V; normalization is
applied to the 128-wide per-head output ahead of the out projection.

Layout (per core), everything E/K-major for the PE:
  xT  [E, N]   = x[b].T          kT [128d, N]    scoresT [s, n] chunks
  wq  [E, 512] = Wq rows.T       qT [128, 4g, N]
  wk  [E, 128] = Wk rows.T       v  [128, 16st, 130] (col 128 = ones)
  wv  [E, 128]                   oT [128, 4g, N]
  wo  [512, E] = Wo cols.T       out [N, E] bf16 partial

Schedule notes (from NTFF profiling):
 - input DMAs are issued in consumption order on separate queues so the
   first k matmul can start ~11us instead of ~25us;
 - dummy matmuls on the identity ramp the PE out of its low p-state
   while the first x chunk is in flight;
 - o-groups trail their chunk's exps by >=2 score-pairs so the PE never
   waits on the ACT engine; out-projection n-tiles are spread one per
   sub-slot instead of 4-tile bursts (P1/DVE ring pressure);
 - out DMAs are issued from gpsimd (keeps the ACT engine exp-only).
"""

import sys

sys.path.insert(0, "/opt/trn_rl_repo")

import numpy as np
import ml_dtypes

import concourse.bass as bass
import concourse.mybir as mybir
import concourse.tile as tile
from concourse import bacc
from concourse.bass_utils import run_bass_kernel_spmd
from concourse.masks import make_identity

BF16 = mybir.dt.bfloat16
F32 = mybir.dt.float32
bf16 = ml_dtypes.bfloat16

B, N, E = 2, 2048, 2048
H, D, G = 16, 128, 4
HKV = H // G
JL = G * D                     # 512 local q-head dims per core
ET = E // 128                  # 16
NT = N // 128                  # 16
CH = N // 512                  # 4
SCALE = 1.0 / float(np.sqrt(D))

_cached = {}


def _build(iters=1):
    nc = bacc.Bacc("TRN2", target_bir_lowering=False, debug=False, num_devices=8)

    xT = nc.dram_tensor("xT", [E, N], BF16, kind="ExternalInput")
    wq = nc.dram_tensor("wq", [E, JL], BF16, kind="ExternalInput")
    wk = nc.dram_tensor("wk", [E, D], BF16, kind="ExternalInput")
    wv = nc.dram_tensor("wv", [E, D], BF16, kind="ExternalInput")
    wo = nc.dram_tensor("wo", [JL, E], BF16, kind="ExternalInput")
    out = nc.dram_tensor("out", [N, E], BF16, kind="ExternalOutput")

    with tile.TileContext(nc) as tc:
        with (
            tc.tile_pool(name="const", bufs=1) as cpool,
            tc.tile_pool(name="xp", bufs=1) as xpool,
            tc.tile_pool(name="wp", bufs=1) as wpool,
            tc.tile_pool(name="kvp", bufs=1) as kvpool,
            tc.tile_pool(name="qp", bufs=1) as qpool,
            tc.tile_pool(name="pp", bufs=3) as ppool,
            tc.tile_pool(name="op", bufs=4) as opool,
            tc.tile_pool(name="otp", bufs=1) as otpool,
            tc.tile_pool(name="outp", bufs=4) as outpool,
            tc.tile_pool(name="ps1", bufs=2, space="PSUM") as P1,
            tc.tile_pool(name="ps2", bufs=4, space="PSUM") as P2,
        ):
            ident = cpool.tile([128, 128], BF16, tag="ident")
            make_identity(nc, ident[:])

            for _ in range(iters):
                _emit_iter(nc, tc, ident, xpool, wpool, kvpool, qpool, ppool,
                           opool, otpool, outpool, P1, P2,
                           xT, wq, wk, wv, wo, out)

    nc.compile()
    return nc


def _emit_iter(nc, tc, ident, xpool, wpool, kvpool, qpool, ppool, opool,
               otpool, outpool, P1, P2, xT, wq, wk, wv, wo, out):
    x_sb = xpool.tile([128, ET, N], BF16, tag="x")
    wq_sb = wpool.tile([128, ET, JL], BF16, tag="wq")
    wk_sb = wpool.tile([128, ET, D], BF16, tag="wk")
    wv_sb = wpool.tile([128, ET, D], BF16, tag="wv")
    wo_sb = wpool.tile([128, G, E], BF16, tag="wo")
    kT_sb = kvpool.tile([128, N], BF16, tag="kT")
    v_sb = kvpool.tile([128, NT, 130], BF16, tag="v")
    qT_sb = qpool.tile([128, G, N], BF16, tag="qT")
    oT_sb = otpool.tile([128, G, N], BF16, tag="oT")

    # --- PE warmup: dummy matmuls on the identity while DMAs are in
    # flight, so the PE reaches its full p-state before real work.
    for i in range(24):
        wmt = P2.tile([128, 130], F32, tag="oc", name=f"warm{i}")
        nc.tensor.matmul(wmt[:, 0:128], ident[:], ident[:], start=True, stop=True)

    # --- input DMAs, in consumption order (queues: sync/scalar/gpsimd) ---
    # wk first (2 pieces, sync queue); k's et-loop starts on piece 0.
    wkr = wk.rearrange("(a p) d -> p a d", p=128)
    nc.sync.dma_start(wk_sb[:, 0:8, :], wkr[:, 0:8, :])
    nc.sync.dma_start(wk_sb[:, 8:16, :], wkr[:, 8:16, :])
    # wv on gpsimd ahead of its x share.
    nc.gpsimd.dma_start(wv_sb[:], wv.rearrange("(a p) d -> p a d", p=128))
    # x: 16 et-chunks of 0.5MB over 3 queues, byte-balanced (sync carries
    # wk so it gets fewer), arrival ~= et order.
    xr = xT.rearrange("(a p) n -> p a n", p=128)
    xengs = {0: nc.scalar, 1: nc.gpsimd, 2: nc.scalar, 3: nc.gpsimd,
             4: nc.scalar, 5: nc.gpsimd, 6: nc.sync, 7: nc.scalar,
             8: nc.gpsimd, 9: nc.sync, 10: nc.scalar, 11: nc.gpsimd,
             12: nc.sync, 13: nc.scalar, 14: nc.gpsimd, 15: nc.sync}
    for et in range(ET):
        xengs[et].dma_start(x_sb[:, et, :], xr[:, et, :])
    # wq in 4 per-head pieces behind x on sync: v-tails run before the
    # q-pairs, so piece g only needs to land ~(47 + 10g)us in.
    wqr = wq.rearrange("(a p) j -> p a j", p=128)
    for g in range(G):
        nc.sync.dma_start(wq_sb[:, :, g * 128:(g + 1) * 128],
                          wqr[:, :, g * 128:(g + 1) * 128])
    # wo last on sync (needed ~halfway into attention).
    for jt in range(G):
        nc.sync.dma_start(wo_sb[:, jt, :], wo[jt * 128:(jt + 1) * 128, :])

    nc.vector.memset(v_sb[:, :, 128:129], 1.0)

    # --- phase 1 ---
    # kT: 4 chunk accumulators (2 double-bank P1 tiles) and ALL 16 v
    # s-tile accumulators (4 per P2 bank as [128,128] f32 slices) run
    # concurrently, so the whole k+v projection (28us of PE) trails the
    # x DMA stream (~24us) with no starvation window.
    kp = [P1.tile([128, 1024], F32, tag="mm1024", name=f"kp{_i}") for _i in range(2)]
    kps = [kp[_i // 2][:, (_i % 2) * 512:(_i % 2 + 1) * 512] for _i in range(CH)]
    vacc = [P2.tile([128, 512], F32, tag="oc", name=f"vacc{_i}") for _i in range(4)]
    vps = [vacc[_s // 4][:, (_s % 4) * 128:(_s % 4 + 1) * 128] for _s in range(NT)]
    for et in range(ET):
        for sc in range(CH):
            nc.tensor.matmul(
                kps[sc], wk_sb[:, et, :], x_sb[:, et, sc * 512:(sc + 1) * 512],
                start=(et == 0), stop=(et == ET - 1),
            )
        for st in range(NT):
            nc.tensor.matmul(
                vps[st], x_sb[:, et, st * 128:(st + 1) * 128],
                wv_sb[:, et, :],
                start=(et == 0), stop=(et == ET - 1),
            )
    for sc in range(CH):
        nc.vector.tensor_copy(kT_sb[:, sc * 512:(sc + 1) * 512], kps[sc])
    for st in range(NT):
        nc.vector.tensor_copy(v_sb[:, st, 0:128], vps[st])

    def emit_q_pair(q0, q1):
        ps = P1.tile([128, 1024], F32, tag="mm1024")
        for half, (g, ncg) in enumerate((q0, q1)):
            sl = ps[:, half * 512:(half + 1) * 512]
            for et in range(ET):
                nc.tensor.matmul(
                    sl, wq_sb[:, et, g * 128:(g + 1) * 128],
                    x_sb[:, et, ncg * 512:(ncg + 1) * 512],
                    start=(et == 0), stop=(et == ET - 1),
                )
            nc.vector.tensor_copy(qT_sb[:, g, ncg * 512:(ncg + 1) * 512], sl)

    qlist = [(g, ncg) for g in range(G) for ncg in range(CH)]
    for i in range(8):
        emit_q_pair(qlist[2 * i], qlist[2 * i + 1])

    # --- phase 2 + 3, pipelined per chunk of 512 n-columns ---
    # Scores for two s-tiles share one double-bank psum tile so a single
    # (wider, cheaper per element) Exp covers both. o-groups trail their
    # chunk by >= 2 score-pairs so the PE never catches the ACT engine;
    # one out-projection n-tile is emitted per sub-slot once a column's
    # four heads are done.
    def emit_score_pair(g, c, p_t, sp):
        ps = P1.tile([128, 1024], F32, tag="mm1024")
        for half in range(2):
            st = 2 * sp + half
            nc.tensor.matmul(
                ps[:, half * 512:(half + 1) * 512],
                kT_sb[:, st * 128:(st + 1) * 128],
                qT_sb[:, g, c * 512:(c + 1) * 512],
                start=True, stop=True,
            )
        nc.scalar.activation(
            p_t[:, 2 * sp * 512:(2 * sp + 2) * 512], ps[:],
            mybir.ActivationFunctionType.Exp, scale=SCALE,
        )

    def emit_o_group(g, c, p_t, t):
        pso = P2.tile([128, 130], F32, tag="oc")
        for st in range(NT):
            nc.tensor.matmul(
                pso[:, 0:129], p_t[:, st * 512 + t * 128: st * 512 + (t + 1) * 128],
                v_sb[:, st, 0:129],
                start=(st == 0), stop=(st == NT - 1),
            )
        rc = opool.tile([128, 1], F32, tag="recip")
        nc.vector.reciprocal(rc[:], pso[:, 128:129])
        o_n = opool.tile([128, 128], BF16, tag="o_n")
        nc.vector.tensor_scalar_mul(o_n[:], pso[:, 0:128], rc[:])
        pst = P2.tile([128, 128], BF16, tag="oc")
        nc.tensor.transpose(pst[:], o_n[:], ident[:])
        nc.vector.tensor_copy(
            oT_sb[:, g, c * 512 + t * 128: c * 512 + (t + 1) * 128], pst[:],
        )

    def emit_out_half(nt, half):
        ps = P1.tile([128, 1024], F32, tag="mm1024")
        for e2 in range(2):
            ec = half * 2 + e2
            for g in range(G):
                nc.tensor.matmul(
                    ps[:, e2 * 512:(e2 + 1) * 512],
                    oT_sb[:, g, nt * 128:(nt + 1) * 128],
                    wo_sb[:, g, ec * 512:(ec + 1) * 512],
                    start=(g == 0), stop=(g == G - 1),
                )
        stage = outpool.tile([128, 1024], BF16, tag="out")
        nc.vector.tensor_copy(stage[:], ps[:])
        nc.gpsimd.dma_start(
            out[nt * 128:(nt + 1) * 128, half * 1024:(half + 1) * 1024],
            stage[:],
        )

    # pending o-group / out-half work queue: one entry per "slot" (after
    # each sub's score pairs), two when backed up. Keeps >= 2 pairs of
    # exp lead for o-groups and spreads the out-projection so the P1
    # psum ring and the DVE never gate the PE.
    chunks = [(c, g) for c in range(CH) for g in range(G)]
    pending = []

    def pop_slot():
        if pending:
            pending.pop(0)()
        if len(pending) > 5:
            pending.pop(0)()

    for i, (c, g) in enumerate(chunks):
        p_t = ppool.tile([128, NT * 512], BF16, tag="p", name=f"p{i}")
        for sub in range(4):
            emit_score_pair(g, c, p_t, 2 * sub)
            emit_score_pair(g, c, p_t, 2 * sub + 1)
            pop_slot()
        # queue this chunk's o-groups (consumed over the next chunks);
        # when the column's 4 heads are done, interleave the column's
        # out-projection halves behind the o-groups that produce them.
        og = [lambda g=g, c=c, p_t=p_t, t=t: emit_o_group(g, c, p_t, t)
              for t in range(4)]
        if g == G - 1:
            oh = [lambda nt=4 * c + t, h=h: emit_out_half(nt, h)
                  for t in range(4) for h in range(2)]
            pending.extend([og[0], og[1], og[2], oh[0], og[3], oh[1],
                            oh[2], oh[3], oh[4], oh[5], oh[6], oh[7]])
        else:
            pending.extend(og)
    while pending:
        pop_slot()


def get_nc(iters=1):
    key = ("nc", iters)
    if key not in _cached:
        _cached[key] = _build(iters)
    return _cached[key]


def make_in_maps(x, Wq, Wk, Wv, Wo):
    """Per-core host-side sharding. Core c -> (b=c//4, gk=c%4)."""
    in_maps = []
    xT = [np.ascontiguousarray(x[b].T).astype(bf16) for b in range(B)]
    wq_s = [np.ascontiguousarray(Wq[gk * JL:(gk + 1) * JL, :].T).astype(bf16)
            for gk in range(HKV)]
    wk_s = [np.ascontiguousarray(Wk[gk * D:(gk + 1) * D, :].T).astype(bf16)
            for gk in range(HKV)]
    wv_s = [np.ascontiguousarray(Wv[gk * D:(gk + 1) * D, :].T).astype(bf16)
            for gk in range(HKV)]
    wo_s = [np.ascontiguousarray(Wo[:, gk * JL:(gk + 1) * JL].T).astype(bf16)
            for gk in range(HKV)]
    for c in range(8):
        b, gk = c // 4, c % 4
        in_maps.append({
            "xT": xT[b], "wq": wq_s[gk], "wk": wk_s[gk],
            "wv": wv_s[gk], "wo": wo_s[gk],
        })
    return in_maps


def kernel(x, Wq, Wk, Wv, Wo):
    nc = get_nc()
    in_maps = make_in_maps(x, Wq, Wk, Wv, Wo)
    res = run_bass_kernel_spmd(nc, in_maps, core_ids=list(range(8)))
    out = np.empty((B, N, E), np.float32)
    for b in range(B):
        acc = res.results[b * 4]["out"].astype(np.float32)
        for gk in range(1, HKV):
            acc = acc + res.results[b * 4 + gk]["out"].astype(np.float32)
        out[b] = acc
    return out
